# revision 1
# baseline (speedup 1.0000x reference)
"""ComplexMultiHeadAttention on 8 TRN2 NeuronCores (Bass/Tile).

Problem: B=4, S=1024, D_MODEL=1024, N_HEADS=16, D_HEAD=64, complex-valued
activations stored as a trailing dim of size 2 (real, imag).

    q = to_heads(complex_linear(queries, wq));  k, v likewise
    s_r + i*s_i = (q_r + i q_i)(k_r + i k_i)^T / sqrt(dh)
    a_r = softmax(s_r), a_i = softmax(s_i)      (independent softmaxes)
    o = complex_bmm(a, v);  out = complex_linear(concat_heads(o), wo)

Sharding: head-parallel. Core c owns heads {2c, 2c+1} = 128 contiguous dims
of the hidden axis. Each core computes Q/K/V projections for its 128 output
dims (weights row-sliced), runs attention for its 8 (batch, head) pairs, and
computes a partial O-projection (wo column-sliced on its 128 input dims)
over all 1024 output dims. The host sums the 8 partial outputs — no
on-device collectives.

Layout: tokens always on the FREE dim, features/keys on partitions, so
every matmul is a natural lhsT.T @ rhs with K=128 contraction:
  - inputs passed transposed: x^T [1024 d, 4096 t]
  - projections produce psum [128 outdims, 512 t]; the complex parts are
    handled by accumulating with sign-folded weight copies (w_i, -w_i).
  - scores are computed TRANSPOSED (s^T [k, q]) from Qcat = [q_r; q_i],
    Kcat_r = [k_r; -k_i], Kcat_i = [k_i; k_r] (all [128, S]) — one K=128
    matmul per 128-key chunk, no accumulation.
  - softmax over k (= partitions) skips max-subtraction (scores are O(1)
    by construction, exp cannot overflow) and takes its partition sums
    with a ones[128,128] f32r matmul that replicates Z across partitions,
    making the 1/Z scale an aligned tensor_mul.
  - V is PE-transposed into token-major packs VA=[v_r|v_i], VB=[-v_i|v_r],
    so attn@V accumulates o_pack [o_r|o_i, q] in a single psum group.
All matmuls run in float32r (TF32-like, 1 cycle/row at N=512 — ~4x the
fp32 rate, ~1.5e-4 relative error). fp32r constraint (probed on HW): the
stationary operand must be full M=128; 128-aligned slices are fine.
PSUM accumulates in f32; outputs are f32.
"""

import os
import numpy as np
import ml_dtypes
from contextlib import ExitStack

import concourse.bass as bass
import concourse.tile as tile
from concourse import bacc, mybir

F32 = mybir.dt.float32
F32R = mybir.dt.float32r
BF16 = mybir.dt.bfloat16
EXP = mybir.ActivationFunctionType.Exp

B, S, D, H, DH = 4, 1024, 1024, 16, 64
NCORES = 8
P = 128            # partitions / chunk size
TBLK = 512         # token block (matmul free dim)
DC = D // P        # 8 d-chunks
KC = S // P        # 8 key chunks per batch
HPC = H // NCORES  # 2 heads per core

_CACHE = {}


def _build():
    nc = bacc.Bacc("TRN2", target_bir_lowering=False, debug=False,
                   num_devices=NCORES)

    NT = (B * S) // TBLK
    x_ap = {}
    for t in ("q", "k", "v"):
        for part in ("r", "i"):
            # tiled-contiguous layout: row block (dc*NT + gt)*P : +P is one
            # [128, 512] tile stored contiguously (single-descriptor DMA)
            x_ap[t + part] = nc.dram_tensor(
                f"x{t}_{part}", [DC * NT * P, TBLK],
                BF16 if t == "v" else F32R, kind="ExternalInput").ap()
    # all projections use per-head combined weights: one psum directly
    # produces the attention layout ([q_r;q_i], [k_r;-k_i], [v_r;v_i])
    w_ap = {}
    for t in ("q", "k", "v"):
        for h in range(HPC):
            for suf in ("a", "b"):
                w_ap[f"{t}{suf}{h}"] = nc.dram_tensor(
                    f"w{t}_{suf}{h}", [P, D],
                    BF16 if t == "v" else F32R, kind="ExternalInput").ap()
    wo_ap = {}
    for suf in ("r", "i", "in"):
        wo_ap[suf] = nc.dram_tensor(
            f"wo_{suf}", [P, D], BF16, kind="ExternalInput").ap()
    ident_ap = nc.dram_tensor("ident", [P, P], BF16, kind="ExternalInput").ap()
    ones_ap = nc.dram_tensor("onesin", [P, P], F32R, kind="ExternalInput").ap()
    # same tiled-contiguous trick for outputs: row block (gt*DC + mc)*P
    po_r = nc.dram_tensor("po_r", [NT * DC * P, TBLK], F32,
                          kind="ExternalOutput").ap()
    po_i = nc.dram_tensor("po_i", [NT * DC * P, TBLK], F32,
                          kind="ExternalOutput").ap()

    with tile.TileContext(nc) as tc, ExitStack() as ctx:
        wpool = ctx.enter_context(tc.tile_pool(name="w", bufs=1))
        xpool = ctx.enter_context(tc.tile_pool(name="x", bufs=12))
        qkpool = ctx.enter_context(tc.tile_pool(name="qk", bufs=2))
        vpool = ctx.enter_context(tc.tile_pool(name="v", bufs=2))
        opool = ctx.enter_context(tc.tile_pool(name="ost", bufs=2))
        upool = ctx.enter_context(tc.tile_pool(name="u", bufs=6))
        zpool = ctx.enter_context(tc.tile_pool(name="z", bufs=2))
        tmppool = ctx.enter_context(tc.tile_pool(name="tmp", bufs=4))
        popool = ctx.enter_context(tc.tile_pool(name="po", bufs=4))
        vstpool = ctx.enter_context(tc.tile_pool(name="vst", bufs=2))
        # PSUM: 8 banks total. projps doubles as the V-transpose target;
        # sps doubles as the O-projection accumulator (same tag).
        projps = ctx.enter_context(tc.tile_pool(name="pp", bufs=2, space="PSUM"))
        sps = ctx.enter_context(tc.tile_pool(name="sp", bufs=2, space="PSUM"))
        zps_pool = ctx.enter_context(tc.tile_pool(name="zp", bufs=1, space="PSUM"))
        ops_pool = ctx.enter_context(tc.tile_pool(name="op", bufs=1, space="PSUM"))

        wt = {}
        for key, ap in list(w_ap.items()):
            wdt = BF16 if key.startswith("v") else F32R
            wt[key] = wpool.tile([P, D], wdt, tag=f"w_{key}", name=f"w_{key}")
            nc.sync.dma_start(wt[key][:], ap[:])
        wot = {}
        for suf, ap in wo_ap.items():
            wot[suf] = wpool.tile([P, D], BF16, tag=f"wo_{suf}",
                                  name=f"wo_{suf}")
            nc.sync.dma_start(wot[suf][:], ap[:])
        ident = wpool.tile([P, P], BF16, tag="ident", name="ident")
        nc.sync.dma_start(ident[:], ident_ap[:])
        ones = wpool.tile([P, P], F32R, tag="ones", name="ones")
        nc.sync.dma_start(ones[:], ones_ap[:])

        for b in range(B):
            qcat = [qkpool.tile([P, S], F32R, tag=f"qcat{h}", name=f"qcat{h}")
                    for h in range(HPC)]
            kcr = [qkpool.tile([P, S], F32R, tag=f"kcr{h}", name=f"kcr{h}")
                   for h in range(HPC)]
            kci = [qkpool.tile([P, S], F32R, tag=f"kci{h}", name=f"kci{h}")
                   for h in range(HPC)]
            va = [vpool.tile([P, S], F32R, tag=f"va{h}", name=f"va{h}")
                  for h in range(HPC)]
            vb = [vpool.tile([P, S], F32R, tag=f"vb{h}", name=f"vb{h}")
                  for h in range(HPC)]
            o_stage = {p: opool.tile([P, S], BF16, tag=f"ost{p}",
                                     name=f"ost{p}")
                       for p in ("r", "i")}

            # ---- projections (per token half-block of 512) ----
            NTv = (B * S) // TBLK
            for t in ("q", "k", "v"):
                xdt = BF16 if t == "v" else F32R
                wA = (wt[t + "a0"], wt[t + "a1"])
                wB = (wt[t + "b0"], wt[t + "b1"])
                for half in range(2):
                    gt = 2 * b + half
                    psr = projps.tile([P, TBLK], F32, tag="projps",
                                      name="projps")
                    psi = projps.tile([P, TBLK], F32, tag="projps",
                                      name="projps")
                    for dc in range(DC):
                        ws = slice(dc * P, (dc + 1) * P)
                        r0 = (dc * NTv + gt) * P
                        xrt = xpool.tile([P, TBLK], xdt, tag="xt", name="xt")
                        nc.sync.dma_start(
                            xrt[:], x_ap[t + "r"][r0:r0 + P, :])
                        nc.tensor.matmul(psr[:], wA[0][:, ws], xrt[:],
                                         start=(dc == 0), stop=False)
                        nc.tensor.matmul(psi[:], wA[1][:, ws], xrt[:],
                                         start=(dc == 0), stop=False)
                    for dc in range(DC):
                        ws = slice(dc * P, (dc + 1) * P)
                        r0 = (dc * NTv + gt) * P
                        xit = xpool.tile([P, TBLK], xdt, tag="xt", name="xt")
                        nc.sync.dma_start(
                            xit[:], x_ap[t + "i"][r0:r0 + P, :])
                        nc.tensor.matmul(psr[:], wB[0][:, ws], xit[:],
                                         start=False, stop=(dc == DC - 1))
                        nc.tensor.matmul(psi[:], wB[1][:, ws], xit[:],
                                         start=False, stop=(dc == DC - 1))
                    hs = slice(half * TBLK, (half + 1) * TBLK)
                    if t == "q":
                        # psX = [q_r(h); q_i(h)] = Qcat directly
                        for h, psx in ((0, psr), (1, psi)):
                            nc.vector.tensor_copy(qcat[h][:, hs], psx[:])
                    elif t == "k":
                        # psX = [k_r(h); -k_i(h)] = Kcat_r directly;
                        # Kcat_i = [k_i; k_r] via one negate + one copy
                        for h, psx in ((0, psr), (1, psi)):
                            nc.vector.tensor_copy(kcr[h][:, hs], psx[:])
                            nc.vector.tensor_scalar_mul(kci[h][0:DH, hs],
                                                        psx[DH:P, :], -1.0)
                            nc.vector.tensor_copy(kci[h][DH:P, hs],
                                                  psx[0:DH, :])
                    else:
                        # psr = [v_r(h0); v_i(h0)], psi = [v_r(h1); v_i(h1)]
                        for h, psx in ((0, psr), (1, psi)):
                            vst = vstpool.tile([P, TBLK], BF16, tag="vst",
                                               name="vst")
                            nc.vector.tensor_copy(vst[:], psx[:])
                            ptb = sps.tile([P, TBLK], BF16, tag="sps",
                                           name="ptb")
                            for blk in range(4):
                                bs = slice(blk * P, (blk + 1) * P)
                                nc.tensor.transpose(ptb[:, bs], vst[:, bs],
                                                    ident[:])
                            # ptb cols per blk: [v_r(h) 64 | v_i(h) 64]
                            base = half * TBLK
                            nc.vector.tensor_copy(
                                va[h][:, base:base + TBLK], ptb[:])
                            vbv = vb[h][:, base:base + TBLK].rearrange(
                                "p (k c) -> p k c", c=P)
                            ptv = ptb[:].rearrange("p (k c) -> p k c", c=P)
                            nc.vector.tensor_copy(vbv[:, :, 0:DH],
                                                  ptv[:, :, DH:P])
                            nc.vector.tensor_copy(vbv[:, :, DH:P],
                                                  ptv[:, :, 0:DH])

            # ---- attention for this batch's 2 heads ----
            for h in range(HPC):
                for qb in range(2):
                    qs = slice(qb * TBLK, (qb + 1) * TBLK)
                    zr = zps_pool.tile([P, TBLK], F32, tag="zr", name="zr")
                    zi = zps_pool.tile([P, TBLK], F32, tag="zi", name="zi")
                    ota = ops_pool.tile([P, TBLK], F32, tag="ota", name="ota")
                    otb = ops_pool.tile([P, TBLK], F32, tag="otb", name="otb")
                    for kc in range(KC):
                        ks = slice(kc * P, (kc + 1) * P)
                        first, last = kc == 0, kc == KC - 1
                        str_ = sps.tile([P, TBLK], F32, tag="sps", name="sps")
                        nc.tensor.matmul(str_[:], kcr[h][:, ks],
                                         qcat[h][:, qs], start=True, stop=True)
                        ur = upool.tile([P, TBLK], F32R, tag="u", name="u")
                        nc.scalar.activation(ur[:], str_[:], EXP)
                        sti = sps.tile([P, TBLK], F32, tag="sps", name="sps")
                        nc.tensor.matmul(sti[:], kci[h][:, ks],
                                         qcat[h][:, qs], start=True, stop=True)
                        ui = upool.tile([P, TBLK], F32R, tag="u", name="u")
                        nc.scalar.activation(ui[:], sti[:], EXP)
                        nc.tensor.matmul(zr[:], ones[:], ur[:],
                                         start=first, stop=last)
                        nc.tensor.matmul(zi[:], ones[:], ui[:],
                                         start=first, stop=last)
                        nc.tensor.matmul(ota[:], va[h][:, ks], ur[:],
                                         start=first, stop=last)
                        nc.tensor.matmul(otb[:], vb[h][:, ks], ui[:],
                                         start=first, stop=last)
                    # o_r = (v_r.T u_r)/Z_r - (v_i.T u_i)/Z_i : each AV term
                    # gets its OWN softmax denominator (independent softmaxes)
                    zinv_r = zpool.tile([P, TBLK], F32, tag="zinv", name="zi_r")
                    nc.vector.reciprocal_approx_fast(zinv_r[:], zr[:])
                    zinv_i = zpool.tile([P, TBLK], F32, tag="zinv", name="zi_i")
                    nc.vector.reciprocal_approx_fast(zinv_i[:], zi[:])
                    tmpa = tmppool.tile([P, TBLK], F32, tag="tmp", name="tmpa")
                    nc.vector.tensor_mul(tmpa[:], ota[:], zinv_r[:])
                    tmpb = tmppool.tile([P, TBLK], F32, tag="tmp", name="tmpb")
                    nc.vector.tensor_mul(tmpb[:], otb[:], zinv_i[:])
                    dst = slice(DH * h, DH * (h + 1))
                    nc.vector.tensor_sub(o_stage["r"][dst, qs], tmpa[0:DH, :],
                                         tmpb[0:DH, :])
                    nc.vector.tensor_add(o_stage["i"][dst, qs], tmpa[DH:P, :],
                                         tmpb[DH:P, :])

            # ---- partial O-projection for this batch ----
            for half in range(2):
                hs = slice(half * TBLK, (half + 1) * TBLK)
                gt = 2 * b + half
                for mc in range(DC):
                    ms = slice(mc * P, (mc + 1) * P)
                    orow = (gt * DC + mc) * P
                    gcols = slice(None)
                    pr = sps.tile([P, TBLK], F32, tag="sps", name="ojpr")
                    nc.tensor.matmul(pr[:], wot["r"][:, ms],
                                     o_stage["r"][:, hs],
                                     start=True, stop=False)
                    nc.tensor.matmul(pr[:], wot["in"][:, ms],
                                     o_stage["i"][:, hs],
                                     start=False, stop=True)
                    sbr = popool.tile([P, TBLK], F32, tag="po", name="po")
                    nc.any.tensor_copy(sbr[:], pr[:])
                    nc.sync.dma_start(po_r[orow:orow + P, :], sbr[:])
                    pi = sps.tile([P, TBLK], F32, tag="sps", name="ojpi")
                    nc.tensor.matmul(pi[:], wot["i"][:, ms],
                                     o_stage["r"][:, hs],
                                     start=True, stop=False)
                    nc.tensor.matmul(pi[:], wot["r"][:, ms],
                                     o_stage["i"][:, hs],
                                     start=False, stop=True)
                    sbi = popool.tile([P, TBLK], F32, tag="po", name="po")
                    nc.any.tensor_copy(sbi[:], pi[:])
                    nc.sync.dma_start(po_i[orow:orow + P, :], sbi[:])

    nc.compile()
    return nc


def _w_sbuf_layout(w_t):
    """[D, 128] weight-transpose slice -> SBUF layout [128, dc*128+o]."""
    return np.ascontiguousarray(
        w_t.reshape(DC, P, P).transpose(1, 0, 2).reshape(P, D))


def _tile_x(xT, dtype):
    """[D, B*S] -> tiled-contiguous [DC*NT*P, TBLK] (rows: (dc*NT+gt)*P)."""
    NT = (B * S) // TBLK
    t = xT.reshape(DC, P, NT, TBLK).transpose(0, 2, 1, 3)
    return np.ascontiguousarray(t.reshape(DC * NT * P, TBLK)).astype(dtype)


def _prepare_in_maps(inputs):
    bf = ml_dtypes.bfloat16
    xs = {}
    for name, t in (("queries", "q"), ("keys", "k"), ("values", "v")):
        x = np.asarray(inputs[name], dtype=np.float32)  # [B,S,D,2]
        flat = x.reshape(B * S, D, 2)
        dt_ = bf if t == "v" else np.float32
        xs[t + "r"] = _tile_x(flat[:, :, 0].T, dt_)
        xs[t + "i"] = _tile_x(flat[:, :, 1].T, dt_)

    scale = np.float32(1.0 / np.sqrt(DH))
    in_maps = []
    for c in range(NCORES):
        rows = slice(P * c, P * (c + 1))
        m = {}
        for t in ("q", "k", "v"):
            for part in ("r", "i"):
                m[f"x{t}_{part}"] = xs[t + part]
        for t, wr_name, wi_name in (("q", "wq_r", "wq_i"),
                                    ("k", "wk_r", "wk_i"),
                                    ("v", "wv_r", "wv_i")):
            s = scale if t == "q" else np.float32(1.0)
            wdt = bf if t == "v" else np.float32
            wr = np.asarray(inputs[wr_name], dtype=np.float32)[rows] * s
            wi = np.asarray(inputs[wi_name], dtype=np.float32)[rows] * s
            for h in range(HPC):
                hr = slice(DH * h, DH * (h + 1))
                if t == "q":
                    wa = np.concatenate([wr[hr].T, wi[hr].T], axis=1)
                    wb = np.concatenate([-wi[hr].T, wr[hr].T], axis=1)
                elif t == "k":
                    wa = np.concatenate([wr[hr].T, -wi[hr].T], axis=1)
                    wb = np.concatenate([-wi[hr].T, -wr[hr].T], axis=1)
                else:
                    wa = np.concatenate([wr[hr].T, wi[hr].T], axis=1)
                    wb = np.concatenate([-wi[hr].T, wr[hr].T], axis=1)
                m[f"w{t}_a{h}"] = _w_sbuf_layout(wa).astype(wdt)
                m[f"w{t}_b{h}"] = _w_sbuf_layout(wb).astype(wdt)
        wo_r = np.asarray(inputs["wo_r"], dtype=np.float32)[:, rows]  # [D,128]
        wo_i = np.asarray(inputs["wo_i"], dtype=np.float32)[:, rows]
        m["wo_r"] = np.ascontiguousarray(wo_r.T).astype(bf)  # [128 d, 1024 m]
        m["wo_i"] = np.ascontiguousarray(wo_i.T).astype(bf)
        m["wo_in"] = np.ascontiguousarray(-wo_i.T).astype(bf)
        m["ident"] = np.eye(P, dtype=bf)
        m["onesin"] = np.ones((P, P), dtype=np.float32)
        in_maps.append(m)
    return in_maps


LAST_RESULT = None


def _run(inputs, trace=False):
    global LAST_RESULT
    from concourse.bass_utils import run_bass_kernel_spmd
    if "nc" not in _CACHE:
        _CACHE["nc"] = _build()
    nc = _CACHE["nc"]
    in_maps = _prepare_in_maps(inputs)
    if trace:
        os.environ.pop("BASS_NEVER_TRACE", None)
    else:
        os.environ["BASS_NEVER_TRACE"] = "1"
    res = run_bass_kernel_spmd(nc, in_maps, core_ids=list(range(NCORES)),
                               trace=trace)
    LAST_RESULT = res
    NT = (B * S) // TBLK
    acc_r = np.zeros((NT * DC * P, TBLK), np.float32)
    acc_i = np.zeros((NT * DC * P, TBLK), np.float32)
    for c in range(NCORES):
        acc_r += res.results[c]["po_r"]
        acc_i += res.results[c]["po_i"]

    def untile(po):
        # [NT*DC*P, TBLK] rows (gt*DC+mc)*P -> [D, B*S] -> [B,S,D]
        t = po.reshape(NT, DC, P, TBLK).transpose(1, 2, 0, 3)
        return np.ascontiguousarray(t.reshape(D, B * S)).T.reshape(B, S, D)

    out = np.empty((B, S, D, 2), np.float32)
    out[..., 0] = untile(acc_r)
    out[..., 1] = untile(acc_i)
    return out


def kernel(**inputs):
    return _run(inputs, trace=False)



# revision 4
# speedup vs baseline: 1.1234x; 1.1234x over previous
"""ComplexMultiHeadAttention on 8 TRN2 NeuronCores (Bass/Tile).

Problem: B=4, S=1024, D_MODEL=1024, N_HEADS=16, D_HEAD=64, complex-valued
activations stored as a trailing dim of size 2 (real, imag).

    q = to_heads(complex_linear(queries, wq));  k, v likewise
    s_r + i*s_i = (q_r + i q_i)(k_r + i k_i)^T / sqrt(dh)
    a_r = softmax(s_r), a_i = softmax(s_i)      (independent softmaxes)
    o = complex_bmm(a, v);  out = complex_linear(concat_heads(o), wo)

Sharding: head-parallel. Core c owns heads {2c, 2c+1} = 128 contiguous dims
of the hidden axis. Each core computes Q/K/V projections for its 128 output
dims (weights row-sliced), runs attention for its 8 (batch, head) pairs, and
computes a partial O-projection (wo column-sliced on its 128 input dims)
over all 1024 output dims. The host sums the 8 partial outputs — no
on-device collectives.

Layout: tokens always on the FREE dim, features/keys on partitions, so
every matmul is a natural lhsT.T @ rhs with K=128 contraction:
  - inputs passed transposed: x^T [1024 d, 4096 t]
  - projections produce psum [128 outdims, 512 t]; the complex parts are
    handled by accumulating with sign-folded weight copies (w_i, -w_i).
  - scores are computed TRANSPOSED (s^T [k, q]) from Qcat = [q_r; q_i],
    Kcat_r = [k_r; -k_i], Kcat_i = [k_i; k_r] (all [128, S]) — one K=128
    matmul per 128-key chunk, no accumulation.
  - softmax over k (= partitions) skips max-subtraction (scores are O(1)
    by construction, exp cannot overflow) and takes its partition sums
    with a ones[128,128] f32r matmul that replicates Z across partitions,
    making the 1/Z scale an aligned tensor_mul.
  - V is PE-transposed into token-major packs VA=[v_r|v_i], VB=[-v_i|v_r],
    so attn@V accumulates o_pack [o_r|o_i, q] in a single psum group.
All matmuls run in float32r (TF32-like, 1 cycle/row at N=512 — ~4x the
fp32 rate, ~1.5e-4 relative error). fp32r constraint (probed on HW): the
stationary operand must be full M=128; 128-aligned slices are fine.
PSUM accumulates in f32; partial outputs are written bf16 and summed in
f32 on the host.

Scheduling (engine queues are in-order, so emission order matters):
  per batch: V proj -> [deferred O-proj of prev batch half1] -> K proj ->
  V transposes -> Q proj -> attention (qb0 h0, qb0 h1, qb1 h0) ->
  O-proj half0 -> attention qb1 h1.  Score matmuls run 2 key-chunks ahead
  of the Z/AV accumulation matmuls so the PE never head-of-line blocks on
  the scalar-engine exp latency.  PSUM->SBUF output copies go to the
  otherwise-idle GPSIMD engine.
"""

import os
import numpy as np
import ml_dtypes
from contextlib import ExitStack

import concourse.bass as bass
import concourse.tile as tile
from concourse import bacc, mybir

F32 = mybir.dt.float32
F32R = mybir.dt.float32r
BF16 = mybir.dt.bfloat16
EXP = mybir.ActivationFunctionType.Exp

B, S, D, H, DH = 4, 1024, 1024, 16, 64
NCORES = 8
P = 128            # partitions / chunk size
TBLK = 512         # token block (matmul free dim)
DC = D // P        # 8 d-chunks
KC = S // P        # 8 key chunks per batch
HPC = H // NCORES  # 2 heads per core

_CACHE = {}


def _build():
    nc = bacc.Bacc("TRN2", target_bir_lowering=False, debug=False,
                   num_devices=NCORES)

    NT = (B * S) // TBLK
    x_ap = {}
    for t in ("q", "k", "v"):
        for part in ("r", "i"):
            # tiled-contiguous layout: row block (dc*NT + gt)*P : +P is one
            # [128, 512] tile stored contiguously (single-descriptor DMA)
            x_ap[t + part] = nc.dram_tensor(
                f"x{t}_{part}", [DC * NT * P, TBLK],
                BF16 if t == "v" else F32R, kind="ExternalInput").ap()
    # all projections use per-head combined weights: one psum directly
    # produces the attention layout ([q_r;q_i], [k_r;-k_i], [v_r;v_i])
    w_ap = {}
    for t in ("v", "k", "q"):
        for h in range(HPC):
            for suf in ("a", "b"):
                w_ap[f"{t}{suf}{h}"] = nc.dram_tensor(
                    f"w{t}_{suf}{h}", [P, D],
                    BF16 if t == "v" else F32R, kind="ExternalInput").ap()
    wo_ap = {}
    for suf in ("r", "i", "in"):
        wo_ap[suf] = nc.dram_tensor(
            f"wo_{suf}", [P, D], BF16, kind="ExternalInput").ap()
    ident_ap = nc.dram_tensor("ident", [P, P], BF16, kind="ExternalInput").ap()
    ones_ap = nc.dram_tensor("onesin", [P, P], F32R, kind="ExternalInput").ap()
    # same tiled-contiguous trick for outputs: row block (gt*DC + mc)*P
    po_r = nc.dram_tensor("po_r", [NT * DC * P, TBLK], BF16,
                          kind="ExternalOutput").ap()
    po_i = nc.dram_tensor("po_i", [NT * DC * P, TBLK], BF16,
                          kind="ExternalOutput").ap()

    with tile.TileContext(nc) as tc, ExitStack() as ctx:
        wpool = ctx.enter_context(tc.tile_pool(name="w", bufs=1))
        xpool = ctx.enter_context(tc.tile_pool(name="x", bufs=14))
        qkpool = ctx.enter_context(tc.tile_pool(name="qk", bufs=2))
        vpool = ctx.enter_context(tc.tile_pool(name="v", bufs=2))
        opool = ctx.enter_context(tc.tile_pool(name="ost", bufs=2))
        upool = ctx.enter_context(tc.tile_pool(name="u", bufs=6))
        zpool = ctx.enter_context(tc.tile_pool(name="z", bufs=2))
        tmppool = ctx.enter_context(tc.tile_pool(name="tmp", bufs=3))
        popool = ctx.enter_context(tc.tile_pool(name="po", bufs=4))
        vstpool = ctx.enter_context(tc.tile_pool(name="vst", bufs=4))
        # PSUM: 8 banks total. projps doubles as the V-transpose target;
        # sps doubles as the O-projection accumulator (same tag).
        projps = ctx.enter_context(tc.tile_pool(name="pp", bufs=2, space="PSUM"))
        sps = ctx.enter_context(tc.tile_pool(name="sp", bufs=2, space="PSUM"))
        zps_pool = ctx.enter_context(tc.tile_pool(name="zp", bufs=1, space="PSUM"))
        ops_pool = ctx.enter_context(tc.tile_pool(name="op", bufs=1, space="PSUM"))

        # constants first (V transposes need ident early)
        ident = wpool.tile([P, P], BF16, tag="ident", name="ident")
        nc.sync.dma_start(ident[:], ident_ap[:])
        ones = wpool.tile([P, P], F32R, tag="ones", name="ones")
        nc.sync.dma_start(ones[:], ones_ap[:])
        # preload the exp activation table during startup DMA
        warmt = wpool.tile([1, 1], F32, tag="warmt", name="warmt")
        nc.vector.memset(warmt[:], 0.0)
        warmo = wpool.tile([1, 1], F32, tag="warmo", name="warmo")
        nc.scalar.activation(warmo[:], warmt[:], EXP)

        wt = {}

        def load_w(t):
            for h in range(HPC):
                for suf in ("a", "b"):
                    key = f"{t}{suf}{h}"
                    wdt = BF16 if t == "v" else F32R
                    wt[key] = wpool.tile([P, D], wdt, tag=f"w_{key}",
                                         name=f"w_{key}")
                    nc.sync.dma_start(wt[key][:], w_ap[key][:])

        load_w("v")  # V weights first: V projection is the first phase

        wot = {}

        def load_wo():
            for suf, ap in wo_ap.items():
                wot[suf] = wpool.tile([P, D], BF16, tag=f"wo_{suf}",
                                      name=f"wo_{suf}")
                nc.sync.dma_start(wot[suf][:], ap[:])

        NTv = (B * S) // TBLK

        def proj_mms(t, half, b):
            """Emit the 32 projection matmuls for (t, half); returns psums."""
            xdt = BF16 if t == "v" else F32R
            wA = (wt[t + "a0"], wt[t + "a1"])
            wB = (wt[t + "b0"], wt[t + "b1"])
            gt = 2 * b + half
            psr = projps.tile([P, TBLK], F32, tag="projps", name="projps")
            psi = projps.tile([P, TBLK], F32, tag="projps", name="projps")
            for dc in range(DC):
                ws = slice(dc * P, (dc + 1) * P)
                r0 = (dc * NTv + gt) * P
                xrt = xpool.tile([P, TBLK], xdt, tag="xt", name="xt")
                nc.sync.dma_start(xrt[:], x_ap[t + "r"][r0:r0 + P, :])
                nc.tensor.matmul(psr[:], wA[0][:, ws], xrt[:],
                                 start=(dc == 0), stop=False)
                nc.tensor.matmul(psi[:], wA[1][:, ws], xrt[:],
                                 start=(dc == 0), stop=False)
            for dc in range(DC):
                ws = slice(dc * P, (dc + 1) * P)
                r0 = (dc * NTv + gt) * P
                xit = xpool.tile([P, TBLK], xdt, tag="xt", name="xt")
                nc.sync.dma_start(xit[:], x_ap[t + "i"][r0:r0 + P, :])
                nc.tensor.matmul(psr[:], wB[0][:, ws], xit[:],
                                 start=False, stop=(dc == DC - 1))
                nc.tensor.matmul(psi[:], wB[1][:, ws], xit[:],
                                 start=False, stop=(dc == DC - 1))
            return psr, psi

        def emit_oproj(o_stage, b, half):
            """Partial O-projection for (batch, half): 32 MMs + gpsimd
            copies + DMA out (bf16 partials)."""
            hs = slice(half * TBLK, (half + 1) * TBLK)
            gt = 2 * b + half
            for mc in range(DC):
                ms = slice(mc * P, (mc + 1) * P)
                orow = (gt * DC + mc) * P
                pr = sps.tile([P, TBLK], F32, tag="sps", name="ojpr")
                nc.tensor.matmul(pr[:], wot["r"][:, ms],
                                 o_stage["r"][:, hs], start=True, stop=False)
                nc.tensor.matmul(pr[:], wot["in"][:, ms],
                                 o_stage["i"][:, hs], start=False, stop=True)
                sbr = popool.tile([P, TBLK], BF16, tag="po", name="po")
                nc.scalar.copy(sbr[:], pr[:])
                nc.sync.dma_start(po_r[orow:orow + P, :], sbr[:])
                pi = sps.tile([P, TBLK], F32, tag="sps", name="ojpi")
                nc.tensor.matmul(pi[:], wot["i"][:, ms],
                                 o_stage["r"][:, hs], start=True, stop=False)
                nc.tensor.matmul(pi[:], wot["r"][:, ms],
                                 o_stage["i"][:, hs], start=False, stop=True)
                sbi = popool.tile([P, TBLK], BF16, tag="po", name="po")
                nc.vector.tensor_copy(sbi[:], pi[:])
                nc.sync.dma_start(po_i[orow:orow + P, :], sbi[:])

        def emit_attn_group(qcat, kcr, kci, va, vb, o_stage, h, qb):
            """One (head, 512-query-block) attention group. Score matmuls
            are emitted 2 key-chunks ahead of the Z/AV accumulations."""
            qs = slice(qb * TBLK, (qb + 1) * TBLK)
            zr = zps_pool.tile([P, TBLK], F32, tag="zr", name="zr")
            zi = zps_pool.tile([P, TBLK], F32, tag="zi", name="zi")
            ota = ops_pool.tile([P, TBLK], F32, tag="ota", name="ota")
            otb = ops_pool.tile([P, TBLK], F32, tag="otb", name="otb")
            us = {}

            def emit_scores(kc):
                ks = slice(kc * P, (kc + 1) * P)
                str_ = sps.tile([P, TBLK], F32, tag="sps", name="sps")
                nc.tensor.matmul(str_[:], kcr[h][:, ks], qcat[h][:, qs],
                                 start=True, stop=True)
                ur = upool.tile([P, TBLK], F32R, tag="u", name="u")
                nc.scalar.activation(ur[:], str_[:], EXP)
                sti = sps.tile([P, TBLK], F32, tag="sps", name="sps")
                nc.tensor.matmul(sti[:], kci[h][:, ks], qcat[h][:, qs],
                                 start=True, stop=True)
                ui = upool.tile([P, TBLK], F32R, tag="u", name="u")
                nc.scalar.activation(ui[:], sti[:], EXP)
                us[kc] = (ur, ui)

            emit_scores(0)
            emit_scores(1)
            for kc in range(KC):
                ks = slice(kc * P, (kc + 1) * P)
                first, last = kc == 0, kc == KC - 1
                ur, ui = us.pop(kc)
                nc.tensor.matmul(zr[:], ones[:], ur[:],
                                 start=first, stop=last)
                nc.tensor.matmul(zi[:], ones[:], ui[:],
                                 start=first, stop=last)
                nc.tensor.matmul(ota[:], va[h][:, ks], ur[:],
                                 start=first, stop=last)
                nc.tensor.matmul(otb[:], vb[h][:, ks], ui[:],
                                 start=first, stop=last)
                if kc + 2 < KC:
                    emit_scores(kc + 2)
            # o_r = (v_r.T u_r)/Z_r - (v_i.T u_i)/Z_i : each AV term gets
            # its OWN softmax denominator (independent softmaxes)
            zinv_r = zpool.tile([P, TBLK], F32, tag="zinv", name="zi_r")
            nc.vector.reciprocal_approx_fast(zinv_r[:], zr[:])
            zinv_i = zpool.tile([P, TBLK], F32, tag="zinv", name="zi_i")
            nc.vector.reciprocal_approx_fast(zinv_i[:], zi[:])
            tmpa = tmppool.tile([P, TBLK], F32, tag="tmp", name="tmpa")
            nc.vector.tensor_mul(tmpa[:], ota[:], zinv_r[:])
            tmpb = tmppool.tile([P, TBLK], F32, tag="tmp", name="tmpb")
            nc.vector.tensor_mul(tmpb[:], otb[:], zinv_i[:])
            dst = slice(DH * h, DH * (h + 1))
            nc.vector.tensor_sub(o_stage["r"][dst, qs], tmpa[0:DH, :],
                                 tmpb[0:DH, :])
            nc.vector.tensor_add(o_stage["i"][dst, qs], tmpa[DH:P, :],
                                 tmpb[DH:P, :])

        prev_oproj = None  # (o_stage, b) pending half-1 O-projection

        for b in range(B):
            qcat = [qkpool.tile([P, S], F32R, tag=f"qcat{h}", name=f"qcat{h}")
                    for h in range(HPC)]
            kcr = [qkpool.tile([P, S], F32R, tag=f"kcr{h}", name=f"kcr{h}")
                   for h in range(HPC)]
            kci = [qkpool.tile([P, S], F32R, tag=f"kci{h}", name=f"kci{h}")
                   for h in range(HPC)]
            va = [vpool.tile([P, S], F32R, tag=f"va{h}", name=f"va{h}")
                  for h in range(HPC)]
            vb = [vpool.tile([P, S], F32R, tag=f"vb{h}", name=f"vb{h}")
                  for h in range(HPC)]
            o_stage = {p: opool.tile([P, S], BF16, tag=f"ost{p}",
                                     name=f"ost{p}")
                       for p in ("r", "i")}

            # ---- V projection (both halves); transposes deferred ----
            vsts = []
            for half in range(2):
                psr, psi = proj_mms("v", half, b)
                for h, psx in ((0, psr), (1, psi)):
                    vst = vstpool.tile([P, TBLK], BF16, tag="vst", name="vst")
                    nc.vector.tensor_copy(vst[:], psx[:])
                    vsts.append((h, half, vst))

            # ---- deferred O-projection of previous batch, half 1 ----
            if prev_oproj is not None:
                emit_oproj(prev_oproj[0], prev_oproj[1], 1)
                prev_oproj = None
            if b == 0:
                load_w("k")

            # ---- K projection ----
            for half in range(2):
                psr, psi = proj_mms("k", half, b)
                hs = slice(half * TBLK, (half + 1) * TBLK)
                # psX = [k_r(h); -k_i(h)] = Kcat_r directly;
                # Kcat_i = [k_i; k_r] via one negate + one copy
                for h, psx in ((0, psr), (1, psi)):
                    nc.vector.tensor_copy(kcr[h][:, hs], psx[:])
                    nc.vector.tensor_scalar_mul(kci[h][0:DH, hs],
                                                psx[DH:P, :], -1.0)
                    nc.vector.tensor_copy(kci[h][DH:P, hs], psx[0:DH, :])

            # ---- V transposes (vst tiles ready long ago -> no PE stall) ----
            for h, half, vst in vsts:
                ptb = sps.tile([P, TBLK], BF16, tag="sps", name="ptb")
                for blk in range(4):
                    bs = slice(blk * P, (blk + 1) * P)
                    nc.tensor.transpose(ptb[:, bs], vst[:, bs], ident[:])
                # ptb cols per blk: [v_r(h) 64 | v_i(h) 64]
                base = half * TBLK
                nc.vector.tensor_copy(va[h][:, base:base + TBLK], ptb[:])
                vbv = vb[h][:, base:base + TBLK].rearrange(
                    "p (k c) -> p k c", c=P)
                ptv = ptb[:].rearrange("p (k c) -> p k c", c=P)
                nc.vector.tensor_copy(vbv[:, :, 0:DH], ptv[:, :, DH:P])
                nc.vector.tensor_copy(vbv[:, :, DH:P], ptv[:, :, 0:DH])
            if b == 0:
                load_w("q")

            # ---- Q projection ----
            for half in range(2):
                psr, psi = proj_mms("q", half, b)
                hs = slice(half * TBLK, (half + 1) * TBLK)
                # psX = [q_r(h); q_i(h)] = Qcat directly
                for h, psx in ((0, psr), (1, psi)):
                    nc.vector.tensor_copy(qcat[h][:, hs], psx[:])
            if b == 0:
                load_wo()

            # ---- attention; O-proj half0 hides inside qb1 ----
            emit_attn_group(qcat, kcr, kci, va, vb, o_stage, 0, 0)
            emit_attn_group(qcat, kcr, kci, va, vb, o_stage, 1, 0)
            emit_attn_group(qcat, kcr, kci, va, vb, o_stage, 0, 1)
            emit_oproj(o_stage, b, 0)
            emit_attn_group(qcat, kcr, kci, va, vb, o_stage, 1, 1)
            prev_oproj = (o_stage, b)

        emit_oproj(prev_oproj[0], prev_oproj[1], 1)

    nc.compile()
    return nc


def _w_sbuf_layout(w_t):
    """[D, 128] weight-transpose slice -> SBUF layout [128, dc*128+o]."""
    return np.ascontiguousarray(
        w_t.reshape(DC, P, P).transpose(1, 0, 2).reshape(P, D))


def _tile_x(xT, dtype):
    """[D, B*S] -> tiled-contiguous [DC*NT*P, TBLK] (rows: (dc*NT+gt)*P)."""
    NT = (B * S) // TBLK
    t = xT.reshape(DC, P, NT, TBLK).transpose(0, 2, 1, 3)
    return np.ascontiguousarray(t.reshape(DC * NT * P, TBLK)).astype(dtype)


def _prepare_in_maps(inputs):
    bf = ml_dtypes.bfloat16
    xs = {}
    for name, t in (("queries", "q"), ("keys", "k"), ("values", "v")):
        x = np.asarray(inputs[name], dtype=np.float32)  # [B,S,D,2]
        flat = x.reshape(B * S, D, 2)
        dt_ = bf if t == "v" else np.float32
        xs[t + "r"] = _tile_x(flat[:, :, 0].T, dt_)
        xs[t + "i"] = _tile_x(flat[:, :, 1].T, dt_)

    scale = np.float32(1.0 / np.sqrt(DH))
    in_maps = []
    for c in range(NCORES):
        rows = slice(P * c, P * (c + 1))
        m = {}
        for t in ("q", "k", "v"):
            for part in ("r", "i"):
                m[f"x{t}_{part}"] = xs[t + part]
        for t, wr_name, wi_name in (("q", "wq_r", "wq_i"),
                                    ("k", "wk_r", "wk_i"),
                                    ("v", "wv_r", "wv_i")):
            s = scale if t == "q" else np.float32(1.0)
            wdt = bf if t == "v" else np.float32
            wr = np.asarray(inputs[wr_name], dtype=np.float32)[rows] * s
            wi = np.asarray(inputs[wi_name], dtype=np.float32)[rows] * s
            for h in range(HPC):
                hr = slice(DH * h, DH * (h + 1))
                if t == "q":
                    wa = np.concatenate([wr[hr].T, wi[hr].T], axis=1)
                    wb = np.concatenate([-wi[hr].T, wr[hr].T], axis=1)
                elif t == "k":
                    wa = np.concatenate([wr[hr].T, -wi[hr].T], axis=1)
                    wb = np.concatenate([-wi[hr].T, -wr[hr].T], axis=1)
                else:
                    wa = np.concatenate([wr[hr].T, wi[hr].T], axis=1)
                    wb = np.concatenate([-wi[hr].T, wr[hr].T], axis=1)
                m[f"w{t}_a{h}"] = _w_sbuf_layout(wa).astype(wdt)
                m[f"w{t}_b{h}"] = _w_sbuf_layout(wb).astype(wdt)
        wo_r = np.asarray(inputs["wo_r"], dtype=np.float32)[:, rows]  # [D,128]
        wo_i = np.asarray(inputs["wo_i"], dtype=np.float32)[:, rows]
        m["wo_r"] = np.ascontiguousarray(wo_r.T).astype(bf)  # [128 d, 1024 m]
        m["wo_i"] = np.ascontiguousarray(wo_i.T).astype(bf)
        m["wo_in"] = np.ascontiguousarray(-wo_i.T).astype(bf)
        m["ident"] = np.eye(P, dtype=bf)
        m["onesin"] = np.ones((P, P), dtype=np.float32)
        in_maps.append(m)
    return in_maps


LAST_RESULT = None


def _run(inputs, trace=False):
    global LAST_RESULT
    from concourse.bass_utils import run_bass_kernel_spmd
    if "nc" not in _CACHE:
        _CACHE["nc"] = _build()
    nc = _CACHE["nc"]
    in_maps = _prepare_in_maps(inputs)
    if trace:
        os.environ.pop("BASS_NEVER_TRACE", None)
    else:
        os.environ["BASS_NEVER_TRACE"] = "1"
    res = run_bass_kernel_spmd(nc, in_maps, core_ids=list(range(NCORES)),
                               trace=trace)
    LAST_RESULT = res
    NT = (B * S) // TBLK
    acc_r = np.zeros((NT * DC * P, TBLK), np.float32)
    acc_i = np.zeros((NT * DC * P, TBLK), np.float32)
    for c in range(NCORES):
        acc_r += res.results[c]["po_r"].astype(np.float32)
        acc_i += res.results[c]["po_i"].astype(np.float32)

    def untile(po):
        # [NT*DC*P, TBLK] rows (gt*DC+mc)*P -> [D, B*S] -> [B,S,D]
        t = po.reshape(NT, DC, P, TBLK).transpose(1, 2, 0, 3)
        return np.ascontiguousarray(t.reshape(D, B * S)).T.reshape(B, S, D)

    out = np.empty((B, S, D, 2), np.float32)
    out[..., 0] = untile(acc_r)
    out[..., 1] = untile(acc_i)
    return out


def kernel(**inputs):
    return _run(inputs, trace=False)


# revision 6
# speedup vs baseline: 1.1517x; 1.0252x over previous
"""ComplexMultiHeadAttention on 8 TRN2 NeuronCores (Bass/Tile).

Problem: B=4, S=1024, D_MODEL=1024, N_HEADS=16, D_HEAD=64, complex-valued
activations stored as a trailing dim of size 2 (real, imag).

    q = to_heads(complex_linear(queries, wq));  k, v likewise
    s_r + i*s_i = (q_r + i q_i)(k_r + i k_i)^T / sqrt(dh)
    a_r = softmax(s_r), a_i = softmax(s_i)      (independent softmaxes)
    o = complex_bmm(a, v);  out = complex_linear(concat_heads(o), wo)

Sharding: head-parallel. Core c owns heads {2c, 2c+1} = 128 contiguous dims
of the hidden axis. Each core computes Q/K/V projections for its 128 output
dims (weights row-sliced), runs attention for its 8 (batch, head) pairs, and
computes a partial O-projection (wo column-sliced on its 128 input dims)
over all 1024 output dims. The host sums the 8 partial outputs — no
on-device collectives.

Layout: tokens always on the FREE dim, features/keys on partitions, so
every matmul is a natural lhsT.T @ rhs with K=128 contraction:
  - inputs passed transposed: x^T [1024 d, 4096 t]
  - projections produce psum [128 outdims, 512 t]; the complex parts are
    handled by accumulating with sign-folded weight copies (w_i, -w_i).
  - scores are computed TRANSPOSED (s^T [k, q]) from Qcat = [q_r; q_i],
    Kcat_r = [k_r; -k_i], Kcat_i = [k_i; k_r] (all [128, S]) — one K=128
    matmul per 128-key chunk, no accumulation.
  - softmax over k (= partitions) skips max-subtraction (scores are O(1)
    by construction, exp cannot overflow) and takes its partition sums
    with a ones[128,128] f32r matmul that replicates Z across partitions,
    making the 1/Z scale an aligned tensor_mul.
  - V is PE-transposed into token-major packs VA=[v_r|v_i], VB=[-v_i|v_r],
    so attn@V accumulates o_pack [o_r|o_i, q] in a single psum group.
All matmuls run in float32r (TF32-like, 1 cycle/row at N=512 — ~4x the
fp32 rate, ~1.5e-4 relative error). fp32r constraint (probed on HW): the
stationary operand must be full M=128; 128-aligned slices are fine.
PSUM accumulates in f32; partial outputs are written bf16 and summed in
f32 on the host.

Scheduling (engine queues are in-order, so emission order matters):
  per batch: V proj -> [deferred O-proj of prev batch half1] -> K proj ->
  V transposes -> Q proj -> attention (qb0 h0, qb0 h1, qb1 h0) ->
  O-proj half0 -> attention qb1 h1.  Score matmuls run 2 key-chunks ahead
  of the Z/AV accumulation matmuls so the PE never head-of-line blocks on
  the scalar-engine exp latency.  PSUM->SBUF output copies go to the
  otherwise-idle GPSIMD engine.
"""

import os
import numpy as np
import ml_dtypes
from contextlib import ExitStack

import concourse.bass as bass
import concourse.tile as tile
from concourse import bacc, mybir

F32 = mybir.dt.float32
F32R = mybir.dt.float32r
BF16 = mybir.dt.bfloat16
EXP = mybir.ActivationFunctionType.Exp

B, S, D, H, DH = 4, 1024, 1024, 16, 64
NCORES = 8
P = 128            # partitions / chunk size
TBLK = 512         # token block (matmul free dim)
DC = D // P        # 8 d-chunks
KC = S // P        # 8 key chunks per batch
HPC = H // NCORES  # 2 heads per core

_CACHE = {}


def _build():
    nc = bacc.Bacc("TRN2", target_bir_lowering=False, debug=False,
                   num_devices=NCORES)

    NT = (B * S) // TBLK
    x_ap = {}
    for t in ("q", "k", "v"):
        for part in ("r", "i"):
            # tiled-contiguous layout: row block (dc*NT + gt)*P : +P is one
            # [128, 512] tile stored contiguously (single-descriptor DMA)
            x_ap[t + part] = nc.dram_tensor(
                f"x{t}_{part}", [DC * NT * P, TBLK],
                BF16 if t == "v" else F32R, kind="ExternalInput").ap()
    # all projections use per-head combined weights: one psum directly
    # produces the attention layout ([q_r;q_i], [k_r;-k_i], [v_r;v_i])
    w_ap = {}
    for t in ("v", "k", "q"):
        for h in range(HPC):
            for suf in ("a", "b"):
                w_ap[f"{t}{suf}{h}"] = nc.dram_tensor(
                    f"w{t}_{suf}{h}", [P, D],
                    BF16 if t == "v" else F32R, kind="ExternalInput").ap()
    wo_ap = {}
    for suf in ("r", "i", "in"):
        wo_ap[suf] = nc.dram_tensor(
            f"wo_{suf}", [P, D], BF16, kind="ExternalInput").ap()
    ident_ap = nc.dram_tensor("ident", [P, P], BF16, kind="ExternalInput").ap()
    ones_ap = nc.dram_tensor("onesin", [P, P], F32R, kind="ExternalInput").ap()
    # same tiled-contiguous trick for outputs: row block (gt*DC + mc)*P
    po_r = nc.dram_tensor("po_r", [NT * DC * P, TBLK], BF16,
                          kind="ExternalOutput").ap()
    po_i = nc.dram_tensor("po_i", [NT * DC * P, TBLK], BF16,
                          kind="ExternalOutput").ap()

    with tile.TileContext(nc) as tc, ExitStack() as ctx:
        wpool = ctx.enter_context(tc.tile_pool(name="w", bufs=1))
        xpool = ctx.enter_context(tc.tile_pool(name="x", bufs=14))
        qkpool = ctx.enter_context(tc.tile_pool(name="qk", bufs=2))
        vpool = ctx.enter_context(tc.tile_pool(name="v", bufs=2))
        opool = ctx.enter_context(tc.tile_pool(name="ost", bufs=2))
        upool = ctx.enter_context(tc.tile_pool(name="u", bufs=6))
        zpool = ctx.enter_context(tc.tile_pool(name="z", bufs=2))
        tmppool = ctx.enter_context(tc.tile_pool(name="tmp", bufs=3))
        popool = ctx.enter_context(tc.tile_pool(name="po", bufs=4))
        vstpool = ctx.enter_context(tc.tile_pool(name="vst", bufs=4))
        # PSUM: 8 banks total. projps doubles as the V-transpose target;
        # sps doubles as the O-projection accumulator (same tag).
        projps = ctx.enter_context(tc.tile_pool(name="pp", bufs=2, space="PSUM"))
        sps = ctx.enter_context(tc.tile_pool(name="sp", bufs=2, space="PSUM"))
        zps_pool = ctx.enter_context(tc.tile_pool(name="zp", bufs=1, space="PSUM"))
        ops_pool = ctx.enter_context(tc.tile_pool(name="op", bufs=1, space="PSUM"))

        # constants first (V transposes need ident early)
        ident = wpool.tile([P, P], BF16, tag="ident", name="ident")
        nc.sync.dma_start(ident[:], ident_ap[:])
        ones = wpool.tile([P, P], F32R, tag="ones", name="ones")
        nc.sync.dma_start(ones[:], ones_ap[:])
        # preload the exp activation table during startup DMA
        warmt = wpool.tile([1, 1], F32, tag="warmt", name="warmt")
        nc.vector.memset(warmt[:], 0.0)
        warmo = wpool.tile([1, 1], F32, tag="warmo", name="warmo")
        nc.scalar.activation(warmo[:], warmt[:], EXP)

        wt = {}

        def load_w(t):
            for h in range(HPC):
                for suf in ("a", "b"):
                    key = f"{t}{suf}{h}"
                    wdt = BF16 if t == "v" else F32R
                    wt[key] = wpool.tile([P, D], wdt, tag=f"w_{key}",
                                         name=f"w_{key}")
                    nc.sync.dma_start(wt[key][:], w_ap[key][:])

        load_w("v")  # V weights first: V projection is the first phase

        wot = {}

        def load_wo():
            for suf, ap in wo_ap.items():
                wot[suf] = wpool.tile([P, D], BF16, tag=f"wo_{suf}",
                                      name=f"wo_{suf}")
                nc.sync.dma_start(wot[suf][:], ap[:])

        NTv = (B * S) // TBLK

        def proj_mms(t, half, b):
            """Emit the 32 projection matmuls for (t, half); returns psums."""
            xdt = BF16 if t == "v" else F32R
            wA = (wt[t + "a0"], wt[t + "a1"])
            wB = (wt[t + "b0"], wt[t + "b1"])
            gt = 2 * b + half
            psr = projps.tile([P, TBLK], F32, tag="projps", name="projps")
            psi = projps.tile([P, TBLK], F32, tag="projps", name="projps")
            for dc in range(DC):
                ws = slice(dc * P, (dc + 1) * P)
                r0 = (dc * NTv + gt) * P
                xrt = xpool.tile([P, TBLK], xdt, tag="xt", name="xt")
                nc.sync.dma_start(xrt[:], x_ap[t + "r"][r0:r0 + P, :])
                nc.tensor.matmul(psr[:], wA[0][:, ws], xrt[:],
                                 start=(dc == 0), stop=False)
                nc.tensor.matmul(psi[:], wA[1][:, ws], xrt[:],
                                 start=(dc == 0), stop=False)
            for dc in range(DC):
                ws = slice(dc * P, (dc + 1) * P)
                r0 = (dc * NTv + gt) * P
                xit = xpool.tile([P, TBLK], xdt, tag="xt", name="xt")
                nc.sync.dma_start(xit[:], x_ap[t + "i"][r0:r0 + P, :])
                nc.tensor.matmul(psr[:], wB[0][:, ws], xit[:],
                                 start=False, stop=(dc == DC - 1))
                nc.tensor.matmul(psi[:], wB[1][:, ws], xit[:],
                                 start=False, stop=(dc == DC - 1))
            return psr, psi

        def emit_oproj(o_stage, b, half):
            """Partial O-projection for (batch, half): 32 MMs + gpsimd
            copies + DMA out (bf16 partials)."""
            hs = slice(half * TBLK, (half + 1) * TBLK)
            gt = 2 * b + half
            for mc in range(DC):
                ms = slice(mc * P, (mc + 1) * P)
                orow = (gt * DC + mc) * P
                pr = sps.tile([P, TBLK], F32, tag="sps", name="ojpr")
                nc.tensor.matmul(pr[:], wot["r"][:, ms],
                                 o_stage["r"][:, hs], start=True, stop=False)
                nc.tensor.matmul(pr[:], wot["in"][:, ms],
                                 o_stage["i"][:, hs], start=False, stop=True)
                sbr = popool.tile([P, TBLK], BF16, tag="po", name="po")
                nc.scalar.copy(sbr[:], pr[:])
                nc.sync.dma_start(po_r[orow:orow + P, :], sbr[:])
                pi = sps.tile([P, TBLK], F32, tag="sps", name="ojpi")
                nc.tensor.matmul(pi[:], wot["i"][:, ms],
                                 o_stage["r"][:, hs], start=True, stop=False)
                nc.tensor.matmul(pi[:], wot["r"][:, ms],
                                 o_stage["i"][:, hs], start=False, stop=True)
                sbi = popool.tile([P, TBLK], BF16, tag="po", name="po")
                nc.vector.tensor_copy(sbi[:], pi[:])
                nc.sync.dma_start(po_i[orow:orow + P, :], sbi[:])

        def emit_attn_group(qcat, kcr, kci, va, vb, o_stage, h, qb):
            """One (head, 512-query-block) attention group. Score matmuls
            are emitted 2 key-chunks ahead of the Z/AV accumulations."""
            qs = slice(qb * TBLK, (qb + 1) * TBLK)
            zr = zps_pool.tile([P, TBLK], F32, tag="zr", name="zr")
            zi = zps_pool.tile([P, TBLK], F32, tag="zi", name="zi")
            ota = ops_pool.tile([P, TBLK], F32, tag="ota", name="ota")
            otb = ops_pool.tile([P, TBLK], F32, tag="otb", name="otb")
            us = {}

            def emit_scores(kc):
                ks = slice(kc * P, (kc + 1) * P)
                str_ = sps.tile([P, TBLK], F32, tag="sps", name="sps")
                nc.tensor.matmul(str_[:], kcr[h][:, ks], qcat[h][:, qs],
                                 start=True, stop=True)
                ur = upool.tile([P, TBLK], F32R, tag="u", name="u")
                nc.scalar.activation(ur[:], str_[:], EXP)
                sti = sps.tile([P, TBLK], F32, tag="sps", name="sps")
                nc.tensor.matmul(sti[:], kci[h][:, ks], qcat[h][:, qs],
                                 start=True, stop=True)
                ui = upool.tile([P, TBLK], F32R, tag="u", name="u")
                nc.scalar.activation(ui[:], sti[:], EXP)
                us[kc] = (ur, ui)

            emit_scores(0)
            emit_scores(1)
            for kc in range(KC):
                ks = slice(kc * P, (kc + 1) * P)
                first, last = kc == 0, kc == KC - 1
                ur, ui = us.pop(kc)
                nc.tensor.matmul(zr[:], ones[:], ur[:],
                                 start=first, stop=last)
                nc.tensor.matmul(zi[:], ones[:], ui[:],
                                 start=first, stop=last)
                nc.tensor.matmul(ota[:], va[h][:, ks], ur[:],
                                 start=first, stop=last)
                nc.tensor.matmul(otb[:], vb[h][:, ks], ui[:],
                                 start=first, stop=last)
                if kc + 2 < KC:
                    emit_scores(kc + 2)
            # o_r = (v_r.T u_r)/Z_r - (v_i.T u_i)/Z_i : each AV term gets
            # its OWN softmax denominator (independent softmaxes)
            zinv_r = zpool.tile([P, TBLK], F32, tag="zinv", name="zi_r")
            nc.vector.reciprocal_approx_fast(zinv_r[:], zr[:])
            zinv_i = zpool.tile([P, TBLK], F32, tag="zinv", name="zi_i")
            nc.vector.reciprocal_approx_fast(zinv_i[:], zi[:])
            tmpa = tmppool.tile([P, TBLK], F32, tag="tmp", name="tmpa")
            nc.vector.tensor_mul(tmpa[:], ota[:], zinv_r[:])
            tmpb = tmppool.tile([P, TBLK], F32, tag="tmp", name="tmpb")
            nc.vector.tensor_mul(tmpb[:], otb[:], zinv_i[:])
            dst = slice(DH * h, DH * (h + 1))
            nc.vector.tensor_sub(o_stage["r"][dst, qs], tmpa[0:DH, :],
                                 tmpb[0:DH, :])
            nc.vector.tensor_add(o_stage["i"][dst, qs], tmpa[DH:P, :],
                                 tmpb[DH:P, :])

        class BatchTiles:
            def __init__(self, b):
                self.b = b
                self.qcat = [qkpool.tile([P, S], F32R, tag=f"qcat{h}",
                                         name=f"qcat{h}") for h in range(HPC)]
                self.kcr = [qkpool.tile([P, S], F32R, tag=f"kcr{h}",
                                        name=f"kcr{h}") for h in range(HPC)]
                self.kci = [qkpool.tile([P, S], F32R, tag=f"kci{h}",
                                        name=f"kci{h}") for h in range(HPC)]
                self.va = [vpool.tile([P, S], F32R, tag=f"va{h}",
                                      name=f"va{h}") for h in range(HPC)]
                self.vb = [vpool.tile([P, S], F32R, tag=f"vb{h}",
                                      name=f"vb{h}") for h in range(HPC)]
                self.o_stage = {p: opool.tile([P, S], BF16, tag=f"ost{p}",
                                              name=f"ost{p}")
                                for p in ("r", "i")}
                self.vsts = []

        def emit_vproj(bt, half):
            psr, psi = proj_mms("v", half, bt.b)
            for h, psx in ((0, psr), (1, psi)):
                vst = vstpool.tile([P, TBLK], BF16, tag="vst", name="vst")
                nc.vector.tensor_copy(vst[:], psx[:])
                bt.vsts.append((h, half, vst))

        def emit_kproj(bt, half):
            psr, psi = proj_mms("k", half, bt.b)
            hs = slice(half * TBLK, (half + 1) * TBLK)
            # psX = [k_r(h); -k_i(h)] = Kcat_r directly;
            # Kcat_i = [k_i; k_r] via one negate + one copy
            for h, psx in ((0, psr), (1, psi)):
                nc.vector.tensor_copy(bt.kcr[h][:, hs], psx[:])
                nc.vector.tensor_scalar_mul(bt.kci[h][0:DH, hs],
                                            psx[DH:P, :], -1.0)
                nc.vector.tensor_copy(bt.kci[h][DH:P, hs], psx[0:DH, :])

        def emit_vtrans(bt):
            # vst tiles are long since written -> transposes never stall PE
            for h, half, vst in bt.vsts:
                ptb = sps.tile([P, TBLK], BF16, tag="sps", name="ptb")
                for blk in range(4):
                    bs = slice(blk * P, (blk + 1) * P)
                    nc.tensor.transpose(ptb[:, bs], vst[:, bs], ident[:])
                # ptb cols per blk: [v_r(h) 64 | v_i(h) 64]
                base = half * TBLK
                nc.vector.tensor_copy(bt.va[h][:, base:base + TBLK], ptb[:])
                vbv = bt.vb[h][:, base:base + TBLK].rearrange(
                    "p (k c) -> p k c", c=P)
                ptv = ptb[:].rearrange("p (k c) -> p k c", c=P)
                nc.vector.tensor_copy(vbv[:, :, 0:DH], ptv[:, :, DH:P])
                nc.vector.tensor_copy(vbv[:, :, DH:P], ptv[:, :, 0:DH])

        def emit_qproj(bt, half):
            psr, psi = proj_mms("q", half, bt.b)
            hs = slice(half * TBLK, (half + 1) * TBLK)
            # psX = [q_r(h); q_i(h)] = Qcat directly
            for h, psx in ((0, psr), (1, psi)):
                nc.vector.tensor_copy(bt.qcat[h][:, hs], psx[:])

        def emit_attn(bt, h, qb):
            emit_attn_group(bt.qcat, bt.kcr, bt.kci, bt.va, bt.vb,
                            bt.o_stage, h, qb)

        # ---- prologue: batch 0 projections (DMA-bound warmup) ----
        cur = BatchTiles(0)
        emit_vproj(cur, 0)
        emit_vproj(cur, 1)
        load_w("k")
        emit_kproj(cur, 0)
        emit_kproj(cur, 1)
        load_w("q")
        emit_vtrans(cur)
        emit_qproj(cur, 0)
        emit_qproj(cur, 1)
        load_wo()

        # ---- steady state: attention/O-proj of b interleaved with the ----
        # ---- projections of b+1, so DMA demand is spread evenly       ----
        for b in range(B):
            nxt = BatchTiles(b + 1) if b + 1 < B else None
            emit_attn(cur, 0, 0)
            if nxt:
                emit_vproj(nxt, 0)
            emit_attn(cur, 1, 0)
            if nxt:
                emit_vproj(nxt, 1)
            emit_attn(cur, 0, 1)
            if nxt:
                emit_kproj(nxt, 0)
            emit_oproj(cur.o_stage, b, 0)
            if nxt:
                emit_kproj(nxt, 1)
            emit_attn(cur, 1, 1)
            if nxt:
                emit_vtrans(nxt)
            emit_oproj(cur.o_stage, b, 1)
            if nxt:
                emit_qproj(nxt, 0)
                emit_qproj(nxt, 1)
            cur = nxt

    nc.compile()
    return nc


def _w_sbuf_layout(w_t):
    """[D, 128] weight-transpose slice -> SBUF layout [128, dc*128+o]."""
    return np.ascontiguousarray(
        w_t.reshape(DC, P, P).transpose(1, 0, 2).reshape(P, D))


def _tile_x(xT, dtype):
    """[D, B*S] -> tiled-contiguous [DC*NT*P, TBLK] (rows: (dc*NT+gt)*P)."""
    NT = (B * S) // TBLK
    t = xT.reshape(DC, P, NT, TBLK).transpose(0, 2, 1, 3)
    return np.ascontiguousarray(t.reshape(DC * NT * P, TBLK)).astype(dtype)


def _prepare_in_maps(inputs):
    bf = ml_dtypes.bfloat16
    xs = {}
    for name, t in (("queries", "q"), ("keys", "k"), ("values", "v")):
        x = np.asarray(inputs[name], dtype=np.float32)  # [B,S,D,2]
        flat = x.reshape(B * S, D, 2)
        dt_ = bf if t == "v" else np.float32
        xs[t + "r"] = _tile_x(flat[:, :, 0].T, dt_)
        xs[t + "i"] = _tile_x(flat[:, :, 1].T, dt_)

    scale = np.float32(1.0 / np.sqrt(DH))
    in_maps = []
    for c in range(NCORES):
        rows = slice(P * c, P * (c + 1))
        m = {}
        for t in ("q", "k", "v"):
            for part in ("r", "i"):
                m[f"x{t}_{part}"] = xs[t + part]
        for t, wr_name, wi_name in (("q", "wq_r", "wq_i"),
                                    ("k", "wk_r", "wk_i"),
                                    ("v", "wv_r", "wv_i")):
            s = scale if t == "q" else np.float32(1.0)
            wdt = bf if t == "v" else np.float32
            wr = np.asarray(inputs[wr_name], dtype=np.float32)[rows] * s
            wi = np.asarray(inputs[wi_name], dtype=np.float32)[rows] * s
            for h in range(HPC):
                hr = slice(DH * h, DH * (h + 1))
                if t == "q":
                    wa = np.concatenate([wr[hr].T, wi[hr].T], axis=1)
                    wb = np.concatenate([-wi[hr].T, wr[hr].T], axis=1)
                elif t == "k":
                    wa = np.concatenate([wr[hr].T, -wi[hr].T], axis=1)
                    wb = np.concatenate([-wi[hr].T, -wr[hr].T], axis=1)
                else:
                    wa = np.concatenate([wr[hr].T, wi[hr].T], axis=1)
                    wb = np.concatenate([-wi[hr].T, wr[hr].T], axis=1)
                m[f"w{t}_a{h}"] = _w_sbuf_layout(wa).astype(wdt)
                m[f"w{t}_b{h}"] = _w_sbuf_layout(wb).astype(wdt)
        wo_r = np.asarray(inputs["wo_r"], dtype=np.float32)[:, rows]  # [D,128]
        wo_i = np.asarray(inputs["wo_i"], dtype=np.float32)[:, rows]
        m["wo_r"] = np.ascontiguousarray(wo_r.T).astype(bf)  # [128 d, 1024 m]
        m["wo_i"] = np.ascontiguousarray(wo_i.T).astype(bf)
        m["wo_in"] = np.ascontiguousarray(-wo_i.T).astype(bf)
        m["ident"] = np.eye(P, dtype=bf)
        m["onesin"] = np.ones((P, P), dtype=np.float32)
        in_maps.append(m)
    return in_maps


LAST_RESULT = None


def _run(inputs, trace=False):
    global LAST_RESULT
    from concourse.bass_utils import run_bass_kernel_spmd
    if "nc" not in _CACHE:
        _CACHE["nc"] = _build()
    nc = _CACHE["nc"]
    in_maps = _prepare_in_maps(inputs)
    if trace:
        os.environ.pop("BASS_NEVER_TRACE", None)
    else:
        os.environ["BASS_NEVER_TRACE"] = "1"
    res = run_bass_kernel_spmd(nc, in_maps, core_ids=list(range(NCORES)),
                               trace=trace)
    LAST_RESULT = res
    NT = (B * S) // TBLK
    acc_r = np.zeros((NT * DC * P, TBLK), np.float32)
    acc_i = np.zeros((NT * DC * P, TBLK), np.float32)
    for c in range(NCORES):
        acc_r += res.results[c]["po_r"].astype(np.float32)
        acc_i += res.results[c]["po_i"].astype(np.float32)

    def untile(po):
        # [NT*DC*P, TBLK] rows (gt*DC+mc)*P -> [D, B*S] -> [B,S,D]
        t = po.reshape(NT, DC, P, TBLK).transpose(1, 2, 0, 3)
        return np.ascontiguousarray(t.reshape(D, B * S)).T.reshape(B, S, D)

    out = np.empty((B, S, D, 2), np.float32)
    out[..., 0] = untile(acc_r)
    out[..., 1] = untile(acc_i)
    return out


def kernel(**inputs):
    return _run(inputs, trace=False)


# revision 7
# speedup vs baseline: 1.3255x; 1.1509x over previous
"""ComplexMultiHeadAttention on 8 TRN2 NeuronCores (Bass/Tile).

Problem: B=4, S=1024, D_MODEL=1024, N_HEADS=16, D_HEAD=64, complex-valued
activations stored as a trailing dim of size 2 (real, imag).

    q = to_heads(complex_linear(queries, wq));  k, v likewise
    s_r + i*s_i = (q_r + i q_i)(k_r + i k_i)^T / sqrt(dh)
    a_r = softmax(s_r), a_i = softmax(s_i)      (independent softmaxes)
    o = complex_bmm(a, v);  out = complex_linear(concat_heads(o), wo)

Sharding: head-parallel. Core c owns heads {2c, 2c+1} = 128 contiguous dims
of the hidden axis. Each core computes Q/K/V projections for its 128 output
dims (weights row-sliced), runs attention for its 8 (batch, head) pairs, and
computes a partial O-projection (wo column-sliced on its 128 input dims)
over all 1024 output dims. The host sums the 8 partial outputs — no
on-device collectives.

Layout: tokens always on the FREE dim, features/keys on partitions, so
every matmul is a natural lhsT.T @ rhs with K=128 contraction:
  - inputs passed transposed: x^T [1024 d, 4096 t]
  - projections produce psum [128 outdims, 512 t]; the complex parts are
    handled by accumulating with sign-folded weight copies (w_i, -w_i).
  - scores are computed TRANSPOSED (s^T [k, q]) from Qcat = [q_r; q_i],
    Kcat_r = [k_r; -k_i], Kcat_i = [k_i; k_r] (all [128, S]) — one K=128
    matmul per 128-key chunk, no accumulation.
  - softmax over k (= partitions) skips max-subtraction (scores are O(1)
    by construction, exp cannot overflow) and takes its partition sums
    with a ones[128,128] f32r matmul that replicates Z across partitions,
    making the 1/Z scale an aligned tensor_mul.
  - V is PE-transposed into token-major packs VA=[v_r|v_i], VB=[-v_i|v_r],
    so attn@V accumulates o_pack [o_r|o_i, q] in a single psum group.
All matmuls run in float32r (TF32-like, 1 cycle/row at N=512 — ~4x the
fp32 rate, ~1.5e-4 relative error). fp32r constraint (probed on HW): the
stationary operand must be full M=128; 128-aligned slices are fine.
PSUM accumulates in f32; partial outputs are written bf16 and summed in
f32 on the host.

Scheduling (engine queues are in-order, so emission order matters):
  per batch: V proj -> [deferred O-proj of prev batch half1] -> K proj ->
  V transposes -> Q proj -> attention (qb0 h0, qb0 h1, qb1 h0) ->
  O-proj half0 -> attention qb1 h1.  Score matmuls run 2 key-chunks ahead
  of the Z/AV accumulation matmuls so the PE never head-of-line blocks on
  the scalar-engine exp latency.  PSUM->SBUF output copies go to the
  otherwise-idle GPSIMD engine.
"""

import os
import numpy as np
import ml_dtypes
from contextlib import ExitStack

import concourse.bass as bass
import concourse.tile as tile
from concourse import bacc, mybir

F32 = mybir.dt.float32
F32R = mybir.dt.float32r
BF16 = mybir.dt.bfloat16
EXP = mybir.ActivationFunctionType.Exp

B, S, D, H, DH = 4, 1024, 1024, 16, 64
NCORES = 8
P = 128            # partitions / chunk size
TBLK = 512         # token block (matmul free dim)
DC = D // P        # 8 d-chunks
KC = S // P        # 8 key chunks per batch
HPC = H // NCORES  # 2 heads per core

_CACHE = {}


def _build():
    nc = bacc.Bacc("TRN2", target_bir_lowering=False, debug=False,
                   num_devices=NCORES)

    NT = (B * S) // TBLK
    x_ap = {}
    for t in ("q", "k", "v"):
        for part in ("r", "i"):
            # tiled-contiguous layout: row block (dc*NT + gt)*P : +P is one
            # [128, 512] tile stored contiguously (single-descriptor DMA)
            x_ap[t + part] = nc.dram_tensor(
                f"x{t}_{part}", [DC * NT * P, TBLK],
                BF16 if t == "v" else F32R, kind="ExternalInput").ap()
    # all projections use per-head combined weights: one psum directly
    # produces the attention layout ([q_r;q_i], [k_r;-k_i], [v_r;v_i])
    w_ap = {}
    for t in ("v", "k", "q"):
        for h in range(HPC):
            for suf in ("a", "b"):
                w_ap[f"{t}{suf}{h}"] = nc.dram_tensor(
                    f"w{t}_{suf}{h}", [P, D],
                    BF16 if t == "v" else F32R, kind="ExternalInput").ap()
    wo_ap = {}
    for suf in ("r", "i", "in"):
        wo_ap[suf] = nc.dram_tensor(
            f"wo_{suf}", [P, D], BF16, kind="ExternalInput").ap()
    ident_ap = nc.dram_tensor("ident", [P, P], BF16, kind="ExternalInput").ap()
    ones_ap = nc.dram_tensor("onesin", [P, P], BF16, kind="ExternalInput").ap()
    # same tiled-contiguous trick for outputs: row block (gt*DC + mc)*P
    po_r = nc.dram_tensor("po_r", [NT * DC * P, TBLK], BF16,
                          kind="ExternalOutput").ap()
    po_i = nc.dram_tensor("po_i", [NT * DC * P, TBLK], BF16,
                          kind="ExternalOutput").ap()

    with tile.TileContext(nc) as tc, ExitStack() as ctx:
        wpool = ctx.enter_context(tc.tile_pool(name="w", bufs=1))
        xpool = ctx.enter_context(tc.tile_pool(name="x", bufs=16))
        qkpool = ctx.enter_context(tc.tile_pool(name="qk", bufs=2))
        vpool = ctx.enter_context(tc.tile_pool(name="v", bufs=2))
        opool = ctx.enter_context(tc.tile_pool(name="ost", bufs=2))
        upool = ctx.enter_context(tc.tile_pool(name="u", bufs=6))
        zpool = ctx.enter_context(tc.tile_pool(name="z", bufs=2))
        tmppool = ctx.enter_context(tc.tile_pool(name="tmp", bufs=3))
        popool = ctx.enter_context(tc.tile_pool(name="po", bufs=4))
        vstpool = ctx.enter_context(tc.tile_pool(name="vst", bufs=4))
        fpool = ctx.enter_context(tc.tile_pool(name="fac", bufs=4))
        # PSUM: 8 banks total. projps doubles as the V-transpose target;
        # sps doubles as the O-projection accumulator (same tag).
        projps = ctx.enter_context(tc.tile_pool(name="pp", bufs=3, space="PSUM"))
        sps = ctx.enter_context(tc.tile_pool(name="sp", bufs=3, space="PSUM"))
        ops_pool = ctx.enter_context(tc.tile_pool(name="op", bufs=1, space="PSUM"))

        # constants first (V transposes need ident early)
        ident = wpool.tile([P, P], BF16, tag="ident", name="ident")
        nc.sync.dma_start(ident[:], ident_ap[:])
        ones = wpool.tile([P, P], BF16, tag="ones", name="ones")
        nc.sync.dma_start(ones[:], ones_ap[:])
        # preload the exp activation table during startup DMA
        warmt = wpool.tile([1, 1], F32, tag="warmt", name="warmt")
        nc.vector.memset(warmt[:], 0.0)
        warmo = wpool.tile([1, 1], F32, tag="warmo", name="warmo")
        nc.scalar.activation(warmo[:], warmt[:], EXP)

        wt = {}

        def load_w(t):
            for h in range(HPC):
                for suf in ("a", "b"):
                    key = f"{t}{suf}{h}"
                    wdt = BF16 if t == "v" else F32R
                    wt[key] = wpool.tile([P, D], wdt, tag=f"w_{key}",
                                         name=f"w_{key}")
                    nc.sync.dma_start(wt[key][:], w_ap[key][:])

        load_w("v")  # V weights first: V projection is the first phase

        wot = {}

        def load_wo():
            for suf, ap in wo_ap.items():
                wot[suf] = wpool.tile([P, D], BF16, tag=f"wo_{suf}",
                                      name=f"wo_{suf}")
                nc.sync.dma_start(wot[suf][:], ap[:])

        NTv = (B * S) // TBLK

        def proj_mms(t, half, b):
            """Emit the 32 projection matmuls for (t, half); returns psums."""
            xdt = BF16 if t == "v" else F32R
            wA = (wt[t + "a0"], wt[t + "a1"])
            wB = (wt[t + "b0"], wt[t + "b1"])
            gt = 2 * b + half
            psr = projps.tile([P, TBLK], F32, tag="projps", name="projps")
            psi = projps.tile([P, TBLK], F32, tag="projps", name="projps")
            for dc in range(DC):
                ws = slice(dc * P, (dc + 1) * P)
                r0 = (dc * NTv + gt) * P
                xrt = xpool.tile([P, TBLK], xdt, tag="xt", name="xt")
                nc.sync.dma_start(xrt[:], x_ap[t + "r"][r0:r0 + P, :])
                nc.tensor.matmul(psr[:], wA[0][:, ws], xrt[:],
                                 start=(dc == 0), stop=False)
                nc.tensor.matmul(psi[:], wA[1][:, ws], xrt[:],
                                 start=(dc == 0), stop=False)
            for dc in range(DC):
                ws = slice(dc * P, (dc + 1) * P)
                r0 = (dc * NTv + gt) * P
                xit = xpool.tile([P, TBLK], xdt, tag="xt", name="xt")
                nc.sync.dma_start(xit[:], x_ap[t + "i"][r0:r0 + P, :])
                nc.tensor.matmul(psr[:], wB[0][:, ws], xit[:],
                                 start=False, stop=(dc == DC - 1))
                nc.tensor.matmul(psi[:], wB[1][:, ws], xit[:],
                                 start=False, stop=(dc == DC - 1))
            return psr, psi

        def emit_oproj(o_stage, b, half):
            """Partial O-projection for (batch, half): 32 MMs + gpsimd
            copies + DMA out (bf16 partials)."""
            hs = slice(half * TBLK, (half + 1) * TBLK)
            gt = 2 * b + half
            for mc in range(DC):
                ms = slice(mc * P, (mc + 1) * P)
                orow = (gt * DC + mc) * P
                pr = sps.tile([P, TBLK], F32, tag="sps", name="ojpr")
                nc.tensor.matmul(pr[:], wot["r"][:, ms],
                                 o_stage["r"][:, hs], start=True, stop=False)
                nc.tensor.matmul(pr[:], wot["in"][:, ms],
                                 o_stage["i"][:, hs], start=False, stop=True)
                sbr = popool.tile([P, TBLK], BF16, tag="po", name="po")
                nc.scalar.copy(sbr[:], pr[:])
                nc.sync.dma_start(po_r[orow:orow + P, :], sbr[:])
                pi = sps.tile([P, TBLK], F32, tag="sps", name="ojpi")
                nc.tensor.matmul(pi[:], wot["i"][:, ms],
                                 o_stage["r"][:, hs], start=True, stop=False)
                nc.tensor.matmul(pi[:], wot["r"][:, ms],
                                 o_stage["i"][:, hs], start=False, stop=True)
                sbi = popool.tile([P, TBLK], BF16, tag="po", name="po")
                nc.vector.tensor_copy(sbi[:], pi[:])
                nc.sync.dma_start(po_i[orow:orow + P, :], sbi[:])

        def emit_attn_group(qcat, kcr, kci, va, vb, o_stage, h, qb):
            """One (head, 512-query-block) attention group. Score matmuls
            are emitted 2 key-chunks ahead of the AV accumulations; softmax
            denominators are folded on DVE (bf16 tree) with one final
            ones-matmul per part to replicate Z across partitions."""
            qs = slice(qb * TBLK, (qb + 1) * TBLK)
            ota = ops_pool.tile([P, TBLK], F32, tag="ota", name="ota")
            otb = ops_pool.tile([P, TBLK], F32, tag="otb", name="otb")
            us = {}

            def emit_scores(kc):
                ks = slice(kc * P, (kc + 1) * P)
                str_ = sps.tile([P, TBLK], F32, tag="sps", name="sps")
                nc.tensor.matmul(str_[:], kcr[h][:, ks], qcat[h][:, qs],
                                 start=True, stop=True)
                ur = upool.tile([P, TBLK], BF16, tag="u", name="u")
                nc.scalar.activation(ur[:], str_[:], EXP)
                sti = sps.tile([P, TBLK], F32, tag="sps", name="sps")
                nc.tensor.matmul(sti[:], kci[h][:, ks], qcat[h][:, qs],
                                 start=True, stop=True)
                ui = upool.tile([P, TBLK], BF16, tag="u", name="u")
                nc.scalar.activation(ui[:], sti[:], EXP)
                us[kc] = (ur, ui)

            emit_scores(0)
            emit_scores(1)
            acc_r = acc_i = None
            for kc in range(KC):
                ks = slice(kc * P, (kc + 1) * P)
                first, last = kc == 0, kc == KC - 1
                ur, ui = us.pop(kc)
                nc.tensor.matmul(ota[:], va[h][:, ks], ur[:],
                                 start=first, stop=last)
                nc.tensor.matmul(otb[:], vb[h][:, ks], ui[:],
                                 start=first, stop=last)
                if kc == 0:
                    acc_r, acc_i = ur, ui
                else:
                    nr = fpool.tile([P, TBLK], BF16, tag="fac", name="fac")
                    nc.vector.tensor_add(nr[:], acc_r[:], ur[:])
                    ni = fpool.tile([P, TBLK], BF16, tag="fac", name="fac")
                    nc.vector.tensor_add(ni[:], acc_i[:], ui[:])
                    acc_r, acc_i = nr, ni
                if kc + 2 < KC:
                    emit_scores(kc + 2)
            zr = sps.tile([P, TBLK], F32, tag="sps", name="zr")
            nc.tensor.matmul(zr[:], ones[:], acc_r[:], start=True, stop=True)
            zi = sps.tile([P, TBLK], F32, tag="sps", name="zi")
            nc.tensor.matmul(zi[:], ones[:], acc_i[:], start=True, stop=True)
            # o_r = (v_r.T u_r)/Z_r - (v_i.T u_i)/Z_i : each AV term gets
            # its OWN softmax denominator (independent softmaxes)
            zinv_r = zpool.tile([P, TBLK], F32, tag="zinv", name="zi_r")
            nc.vector.reciprocal_approx_fast(zinv_r[:], zr[:])
            zinv_i = zpool.tile([P, TBLK], F32, tag="zinv", name="zi_i")
            nc.vector.reciprocal_approx_fast(zinv_i[:], zi[:])
            tmpa = tmppool.tile([P, TBLK], F32, tag="tmp", name="tmpa")
            nc.vector.tensor_mul(tmpa[:], ota[:], zinv_r[:])
            tmpb = tmppool.tile([P, TBLK], F32, tag="tmp", name="tmpb")
            nc.vector.tensor_mul(tmpb[:], otb[:], zinv_i[:])
            dst = slice(DH * h, DH * (h + 1))
            nc.vector.tensor_sub(o_stage["r"][dst, qs], tmpa[0:DH, :],
                                 tmpb[0:DH, :])
            nc.vector.tensor_add(o_stage["i"][dst, qs], tmpa[DH:P, :],
                                 tmpb[DH:P, :])

        class BatchTiles:
            def __init__(self, b):
                self.b = b
                self.qcat = [qkpool.tile([P, S], F32R, tag=f"qcat{h}",
                                         name=f"qcat{h}") for h in range(HPC)]
                self.kcr = [qkpool.tile([P, S], F32R, tag=f"kcr{h}",
                                        name=f"kcr{h}") for h in range(HPC)]
                self.kci = [qkpool.tile([P, S], F32R, tag=f"kci{h}",
                                        name=f"kci{h}") for h in range(HPC)]
                self.va = [vpool.tile([P, S], BF16, tag=f"va{h}",
                                      name=f"va{h}") for h in range(HPC)]
                self.vb = [vpool.tile([P, S], BF16, tag=f"vb{h}",
                                      name=f"vb{h}") for h in range(HPC)]
                self.o_stage = {p: opool.tile([P, S], BF16, tag=f"ost{p}",
                                              name=f"ost{p}")
                                for p in ("r", "i")}
                self.vsts = []

        def emit_vproj(bt, half):
            psr, psi = proj_mms("v", half, bt.b)
            for h, psx in ((0, psr), (1, psi)):
                vst = vstpool.tile([P, TBLK], BF16, tag="vst", name="vst")
                nc.vector.tensor_copy(vst[:], psx[:])
                bt.vsts.append((h, half, vst))

        def emit_kproj(bt, half):
            psr, psi = proj_mms("k", half, bt.b)
            hs = slice(half * TBLK, (half + 1) * TBLK)
            # psX = [k_r(h); -k_i(h)] = Kcat_r directly;
            # Kcat_i = [k_i; k_r] via one negate + one copy
            for h, psx in ((0, psr), (1, psi)):
                nc.scalar.copy(bt.kcr[h][:, hs], psx[:])
                nc.vector.tensor_scalar_mul(bt.kci[h][0:DH, hs],
                                            psx[DH:P, :], -1.0)
                nc.vector.tensor_copy(bt.kci[h][DH:P, hs], psx[0:DH, :])

        def emit_vtrans(bt):
            # vst tiles are long since written -> transposes never stall PE
            for h, half, vst in bt.vsts:
                ptb = sps.tile([P, TBLK], BF16, tag="sps", name="ptb")
                for blk in range(4):
                    bs = slice(blk * P, (blk + 1) * P)
                    nc.tensor.transpose(ptb[:, bs], vst[:, bs], ident[:])
                # ptb cols per blk: [v_r(h) 64 | v_i(h) 64]
                base = half * TBLK
                nc.vector.tensor_copy(bt.va[h][:, base:base + TBLK], ptb[:])
                vbv = bt.vb[h][:, base:base + TBLK].rearrange(
                    "p (k c) -> p k c", c=P)
                ptv = ptb[:].rearrange("p (k c) -> p k c", c=P)
                nc.vector.tensor_copy(vbv[:, :, 0:DH], ptv[:, :, DH:P])
                nc.vector.tensor_copy(vbv[:, :, DH:P], ptv[:, :, 0:DH])

        def emit_qproj(bt, half):
            psr, psi = proj_mms("q", half, bt.b)
            hs = slice(half * TBLK, (half + 1) * TBLK)
            # psX = [q_r(h); q_i(h)] = Qcat directly
            for h, psx in ((0, psr), (1, psi)):
                nc.scalar.copy(bt.qcat[h][:, hs], psx[:])

        def emit_attn(bt, h, qb):
            emit_attn_group(bt.qcat, bt.kcr, bt.kci, bt.va, bt.vb,
                            bt.o_stage, h, qb)

        # ---- prologue: batch 0 projections (DMA-bound warmup) ----
        cur = BatchTiles(0)
        emit_vproj(cur, 0)
        emit_vproj(cur, 1)
        load_w("k")
        emit_kproj(cur, 0)
        emit_kproj(cur, 1)
        load_w("q")
        emit_vtrans(cur)
        emit_qproj(cur, 0)
        emit_qproj(cur, 1)
        load_wo()

        # ---- steady state: attention/O-proj of b interleaved with the ----
        # ---- projections of b+1, so DMA demand is spread evenly       ----
        for b in range(B):
            nxt = BatchTiles(b + 1) if b + 1 < B else None
            emit_attn(cur, 0, 0)
            if nxt:
                emit_vproj(nxt, 0)
            emit_attn(cur, 1, 0)
            if nxt:
                emit_vproj(nxt, 1)
            emit_attn(cur, 0, 1)
            if nxt:
                emit_kproj(nxt, 0)
            emit_oproj(cur.o_stage, b, 0)
            if nxt:
                emit_kproj(nxt, 1)
            emit_attn(cur, 1, 1)
            if nxt:
                emit_vtrans(nxt)
            emit_oproj(cur.o_stage, b, 1)
            if nxt:
                emit_qproj(nxt, 0)
                emit_qproj(nxt, 1)
            cur = nxt

    nc.compile()
    return nc


def _w_sbuf_layout(w_t):
    """[D, 128] weight-transpose slice -> SBUF layout [128, dc*128+o]."""
    return np.ascontiguousarray(
        w_t.reshape(DC, P, P).transpose(1, 0, 2).reshape(P, D))


def _tile_x(xT, dtype):
    """[D, B*S] -> tiled-contiguous [DC*NT*P, TBLK] (rows: (dc*NT+gt)*P)."""
    NT = (B * S) // TBLK
    t = xT.reshape(DC, P, NT, TBLK).transpose(0, 2, 1, 3)
    return np.ascontiguousarray(t.reshape(DC * NT * P, TBLK)).astype(dtype)


def _prepare_in_maps(inputs):
    bf = ml_dtypes.bfloat16
    xs = {}
    for name, t in (("queries", "q"), ("keys", "k"), ("values", "v")):
        x = np.asarray(inputs[name], dtype=np.float32)  # [B,S,D,2]
        flat = x.reshape(B * S, D, 2)
        dt_ = bf if t == "v" else np.float32
        xs[t + "r"] = _tile_x(flat[:, :, 0].T, dt_)
        xs[t + "i"] = _tile_x(flat[:, :, 1].T, dt_)

    scale = np.float32(1.0 / np.sqrt(DH))
    in_maps = []
    for c in range(NCORES):
        rows = slice(P * c, P * (c + 1))
        m = {}
        for t in ("q", "k", "v"):
            for part in ("r", "i"):
                m[f"x{t}_{part}"] = xs[t + part]
        for t, wr_name, wi_name in (("q", "wq_r", "wq_i"),
                                    ("k", "wk_r", "wk_i"),
                                    ("v", "wv_r", "wv_i")):
            s = scale if t == "q" else np.float32(1.0)
            wdt = bf if t == "v" else np.float32
            wr = np.asarray(inputs[wr_name], dtype=np.float32)[rows] * s
            wi = np.asarray(inputs[wi_name], dtype=np.float32)[rows] * s
            for h in range(HPC):
                hr = slice(DH * h, DH * (h + 1))
                if t == "q":
                    wa = np.concatenate([wr[hr].T, wi[hr].T], axis=1)
                    wb = np.concatenate([-wi[hr].T, wr[hr].T], axis=1)
                elif t == "k":
                    wa = np.concatenate([wr[hr].T, -wi[hr].T], axis=1)
                    wb = np.concatenate([-wi[hr].T, -wr[hr].T], axis=1)
                else:
                    wa = np.concatenate([wr[hr].T, wi[hr].T], axis=1)
                    wb = np.concatenate([-wi[hr].T, wr[hr].T], axis=1)
                m[f"w{t}_a{h}"] = _w_sbuf_layout(wa).astype(wdt)
                m[f"w{t}_b{h}"] = _w_sbuf_layout(wb).astype(wdt)
        wo_r = np.asarray(inputs["wo_r"], dtype=np.float32)[:, rows]  # [D,128]
        wo_i = np.asarray(inputs["wo_i"], dtype=np.float32)[:, rows]
        m["wo_r"] = np.ascontiguousarray(wo_r.T).astype(bf)  # [128 d, 1024 m]
        m["wo_i"] = np.ascontiguousarray(wo_i.T).astype(bf)
        m["wo_in"] = np.ascontiguousarray(-wo_i.T).astype(bf)
        m["ident"] = np.eye(P, dtype=bf)
        m["onesin"] = np.ones((P, P), dtype=bf)
        in_maps.append(m)
    return in_maps


LAST_RESULT = None


def _run(inputs, trace=False):
    global LAST_RESULT
    from concourse.bass_utils import run_bass_kernel_spmd
    if "nc" not in _CACHE:
        _CACHE["nc"] = _build()
    nc = _CACHE["nc"]
    in_maps = _prepare_in_maps(inputs)
    if trace:
        os.environ.pop("BASS_NEVER_TRACE", None)
    else:
        os.environ["BASS_NEVER_TRACE"] = "1"
    res = run_bass_kernel_spmd(nc, in_maps, core_ids=list(range(NCORES)),
                               trace=trace)
    LAST_RESULT = res
    NT = (B * S) // TBLK
    acc_r = np.zeros((NT * DC * P, TBLK), np.float32)
    acc_i = np.zeros((NT * DC * P, TBLK), np.float32)
    for c in range(NCORES):
        acc_r += res.results[c]["po_r"].astype(np.float32)
        acc_i += res.results[c]["po_i"].astype(np.float32)

    def untile(po):
        # [NT*DC*P, TBLK] rows (gt*DC+mc)*P -> [D, B*S] -> [B,S,D]
        t = po.reshape(NT, DC, P, TBLK).transpose(1, 2, 0, 3)
        return np.ascontiguousarray(t.reshape(D, B * S)).T.reshape(B, S, D)

    out = np.empty((B, S, D, 2), np.float32)
    out[..., 0] = untile(acc_r)
    out[..., 1] = untile(acc_i)
    return out


def kernel(**inputs):
    return _run(inputs, trace=False)
